# revision 27
# baseline (speedup 1.0000x reference)
"""Trainium2 Bass kernel for LIFNet (leaky-integrator net, no spiking).

Math: the module is linear, and the leaky integration L (a causal LTI filter
along T) commutes with the per-timestep linear layers:

    V2 = L(L(batch @ W1^T) @ W2^T) = (L^2)(batch @ (W2 @ W1)^T)

with Wc = W2 @ W1 of shape [10, 784].  The double integration is evaluated
EXACTLY as two chained first-order recurrences on the Vector engine's
``tensor_tensor_scan`` (fp32 internal state):

    W[t]  = a*W[t-1]  + b^2 * z[t-1]      (W = b*V1)
    V2[t] = a*V2[t-1] + W[t-1]

so the Tensor engine runs nothing but the z-matmuls, whose 4-way
column-group overlap is preserved (no transpose / filter matmuls to
interleave), and the end-of-stream critical path is two short scan chunks.

Sharding (balanced, max-core bytes minimized): each core gets 12 full b's
(cores 0-7 -> b 12c..12c+11, covering b 0..95) plus HALF (by T) of one of
the remaining b's 96..99: core c processes b 96+c//2, T-half c%2, as a
1152-t segment (128 warm-up t's for the upper half; a^256 << 1e-30 so
starting the recursion 128 t's early is exact to f32).

Device work per core (the stream is HBM-read bound, so x is fp8-e3m4,
host-encoded at 2x scale -- measured end-to-end rel err ~1.3e-2 vs the
2e-2 gate; weights stay bf16, the PE supports mixed bf16 x fp8 exactly):
  - bulk input on the sync HWDGE queue, two 784 KB half-b transfers per b
    ([112 part, 7 KB contiguous lines]); the tiny wct const rides the
    otherwise-empty SWDGE queue concurrently with the first transfer.
  - z^T = Wc @ x^T per 500-t unit: all 7 d-chunks (K=112) accumulate into
    one PSUM band (rows 32q..32q+9 of a [106, 500] bank tile,
    tile_position=(0, 32q), q = unit%4); the four units of a b are
    emitted chunk-outer so LDWEIGHTS at one array position overlaps
    streaming at another and the 4 chains run concurrently.
  - the PSUM band is copied to the z^T staging tile by the Scalar engine
    with a fused *b^2 scale (f32 -> fp16).
  - per band of 4 b's (staging rows 32q..32q+9), the two scans run in
    500-col chunks chained via ``initial=prev[:, c-1:c]``, so each chunk
    fires as soon as the last b's unit-copy lands; V2 rows then DMA out
    per b on the SWDGE queue (empty mid-stream, so writes drain the
    moment they are ready).
  - the last b and the T-segment stream in half-transfers so only ~2
    z-units + two 500-col scan chunks + a 23 KB write remain after the
    last input byte.
  - Host re-assembles [100, 2000, 10].
"""

import sys

import numpy as np

for _p in ("/opt/trn_rl_repo",):
    if _p not in sys.path:
        sys.path.append(_p)

B, T, DIN, H1, H2 = 100, 2000, 784, 100, 10
ALPHA, BETA = 0.7, 0.3

NCORES = 8
BPF = 12            # full b's per core (8 * 12 = 96)
BGRP = 4            # b's per band (32-partition offsets in the staging tile)
DC = 112            # d-chunk width (784 = 7 * 112), partition dim of x tiles
NDC = DIN // DC     # 7
XS = 2.0            # host pre-scale of x before fp8-e3m4 encode
TG = 500            # t-columns per z-matmul unit (PSUM bank max 512 f32)
NTG = T // TG       # 4
TS = 1152           # segment length: 1024 lower / 128 warm-up
SGU = 288           # segment z-matmul unit width (4 * 288 = 1152)
SEG_LO = 1024       # lower-half cores emit t < 1024
SEG_W0 = 128        # upper-half warm-up t's (discarded)

_CACHE: dict = {}


def _build():
    """Build + compile the per-core Bass kernel (shared by all 8 cores)."""
    from contextlib import ExitStack

    import concourse.tile as tile
    from concourse import bacc, mybir

    f32 = mybir.dt.float32
    bf16 = mybir.dt.bfloat16
    fp16 = mybir.dt.float16
    fp8 = mybir.dt.float8e3
    nc = bacc.Bacc(
        "TRN2", target_bir_lowering=False, debug=False, num_devices=NCORES
    )

    xT = nc.dram_tensor(
        "xT", [BPF, 2, DC, NDC, T // 2], fp8, kind="ExternalInput"
    )
    xS = nc.dram_tensor("xS", [2, DC, NDC, TS // 2], fp8, kind="ExternalInput")
    wct = nc.dram_tensor("wct", [DC, NDC * H2], bf16, kind="ExternalInput")
    vout = nc.dram_tensor(
        "vout", [(BPF + 1) * H2, T], fp16, kind="ExternalOutput"
    )

    HF = NDC * (T // 2)  # free offset of t-half 1 in an xt tile

    with tile.TileContext(nc) as tc, ExitStack() as ctx:
        const = ctx.enter_context(tc.tile_pool(name="const", bufs=1))
        xpool = ctx.enter_context(tc.tile_pool(name="xp", bufs=6))
        xspool = ctx.enter_context(tc.tile_pool(name="xs", bufs=2))
        ring = ctx.enter_context(tc.tile_pool(name="ring", bufs=1))
        wpool = ctx.enter_context(tc.tile_pool(name="wp", bufs=2))
        vpool = ctx.enter_context(tc.tile_pool(name="vp", bufs=4))
        zps = ctx.enter_context(tc.tile_pool(name="zps", bufs=2, space="PSUM"))

        # First bulk DMA goes out on the sync HWDGE queue; the wct const
        # rides SWDGE (gpsimd) concurrently.  The SWDGE queue stays
        # empty for the rest of the stream so output writes drain the
        # moment they are ready.
        xt0 = xpool.tile([DC, NDC * T], fp8, tag="xt")
        for h in range(2):
            nc.sync.dma_start(
                xt0[:, h * HF : (h + 1) * HF].rearrange(
                    "p (c t) -> p c t", c=NDC
                ),
                xT.ap()[0, h],
            )
        wct_sb = const.tile([DC, NDC * H2], bf16, tag="wct")
        nc.gpsimd.dma_start(wct_sb[:], wct.ap())

        # alpha operand for the scans (data0 must be a tensor; fp16 so
        # the DVE runs at 2x 16-bit throughput -- the scan state itself
        # stays fp32.  fp16(0.7) shifts the DC gain by ~0.2%, well
        # inside the error budget).
        alpha_sb = const.tile([128, T], fp16, tag="alpha")
        nc.vector.memset(alpha_sb[:], ALPHA)

        # z^T staging ring: bands live at 32-partition offsets (compute
        # engines need 32-aligned partition bases); spare rows are never
        # read to any visible output (the scans are partition-parallel
        # and the out-DMA slices per b), so no zeroing is needed.
        zts_ring = []
        for i in range(2):
            zt = ring.tile([128, T], fp16, tag=f"zts{i}", name=f"zts{i}")
            zts_ring.append(zt)

        def zchains(zts, row0, parts, pos0=0):
            """Interleaved z-matmul unit chains: parts = per-unit
            (xt, xoff, w, toff).  The 7 d-chunks of every unit
            accumulate into PSUM band rows 32q..32q+9 (q = pos0+unit,
            tile_position=(0, 32q)); chunk MMs are emitted c-outer so
            LDWEIGHTS at one array position overlaps streaming at
            another.  Bands are then copied (f32 -> fp16, fused *b^2)
            into the z^T staging tile by the Scalar engine."""
            zp = zps.tile([3 * 32 + H2, TG], f32, tag="zp")
            for c in range(NDC):
                for u, (xt, xoff, w, cs, _) in enumerate(parts):
                    q = pos0 + u
                    nc.tensor.matmul(
                        zp[32 * q : 32 * q + H2, 0:w],
                        wct_sb[:, c * H2 : (c + 1) * H2],
                        xt[:, xoff + c * cs : xoff + c * cs + w],
                        start=(c == 0),
                        stop=(c == NDC - 1),
                        tile_position=(0, 32 * q),
                    )
            for u, (_, _, w, _, toff) in enumerate(parts):
                q = pos0 + u
                nc.scalar.mul(
                    zts[row0 : row0 + H2, toff : toff + w],
                    zp[32 * q : 32 * q + H2, 0:w],
                    BETA * BETA,
                )

        def stage1(b, bq, zts, xt=None):
            if xt is None:
                xt = xpool.tile([DC, NDC * T], fp8, tag="xt")
                for h in range(2):
                    nc.sync.dma_start(
                        xt[:, h * HF : (h + 1) * HF].rearrange(
                            "p (c t) -> p c t", c=NDC
                        ),
                        xT.ap()[b, h],
                    )
            zchains(
                zts, 32 * bq,
                [
                    (xt, (u // 2) * HF + (u % 2) * TG, TG, T // 2, u * TG)
                    for u in range(NTG)
                ],
            )

        def band_scans(bs0, zts, tw, bounds, nb):
            """Two chained scans (W then V2) over the staging tile, in
            chunks so each fires as soon as its z columns land, then the
            per-b output DMAs.  bounds = ascending chunk edges starting
            at 1, ending at tw; nb = b's in the band (rows 32g..32g+9
            hold b bs0+g)."""
            rows = 32 * (nb - 1) + H2
            w = wpool.tile([128, T], fp16, tag="w")
            v2 = vpool.tile([128, T], fp16, tag="v2")
            nc.vector.memset(w[0:rows, 0:1], 0.0)
            nc.vector.memset(v2[0:rows, 0:1], 0.0)
            mult = mybir.AluOpType.mult
            add = mybir.AluOpType.add
            for lo, hi in zip(bounds[:-1], bounds[1:]):
                for s, dd in ((zts, w), (w, v2)):
                    nc.vector.tensor_tensor_scan(
                        dd[0:rows, lo:hi],
                        alpha_sb[0:rows, lo:hi],
                        s[0:rows, lo - 1 : hi - 1],
                        0.0 if lo == 1 else dd[0:rows, lo - 1 : lo],
                        mult,
                        add,
                    )
            def write():
                for g in range(nb):
                    nc.sync.dma_start(
                        vout.ap()[(bs0 + g) * H2 : (bs0 + g + 1) * H2, 0:tw],
                        v2[32 * g : 32 * g + H2, 0:tw],
                    )

            outq.append(write)

        BB = [1, TG + 1, 2 * TG + 1, 3 * TG + 1, T]  # band chunk edges
        outq = []

        # bands 0/1
        for gi in range(2):
            zts = zts_ring[gi % 2]
            for bq in range(BGRP):
                b = gi * BGRP + bq
                stage1(b, bq, zts, xt=xt0 if b == 0 else None)
            band_scans(gi * BGRP, zts, T, BB, BGRP)

        # ---- end-game: band 2 (b8..b11) + segment ----
        zts2 = zts_ring[0]
        ztsS = zts_ring[1]
        for bq, b in enumerate((8, 9, 10)):
            stage1(b, bq, zts2)
        # b11 and the segment stream in half-transfers
        xt11 = []
        for h in range(2):
            xth = xspool.tile([DC, HF], fp8, tag="xth")
            nc.sync.dma_start(
                xth[:].rearrange("p (c t) -> p c t", c=NDC), xT.ap()[11, h]
            )
            xt11.append(xth)
        xh = []
        for h in range(2):
            xts_t = xspool.tile([DC, NDC * (TS // 2)], fp8, tag="xts")
            nc.sync.dma_start(
                xts_t[:].rearrange("p (c t) -> p c t", c=NDC), xS.ap()[h]
            )
            xh.append(xts_t)
        zchains(
            zts2, 32 * 3,
            [(xt11[0], u * TG, TG, T // 2, u * TG) for u in (0, 1)],
            pos0=0,
        )
        zchains(
            zts2, 32 * 3,
            [(xt11[1], (u - 2) * TG, TG, T // 2, u * TG) for u in (2, 3)],
            pos0=2,
        )

        def seg_parts(us):
            return [
                (xh[u // 2], (u % 2) * SGU, SGU, TS // 2, u * SGU) for u in us
            ]

        zchains(ztsS, 0, seg_parts((0, 1)), pos0=0)
        zchains(ztsS, 0, seg_parts((2, 3)), pos0=2)
        band_scans(8, zts2, T, BB, BGRP)
        band_scans(BPF, ztsS, TS, [1, 2 * SGU + 1, TS], 1)
        # all output writes ride the sync HWDGE queue BEHIND the last
        # input transfer: the queue drains them back-to-back the moment
        # each band's V2 is ready, with hardware descriptor generation.
        for write in outq:
            write()

    nc.compile()
    return nc


def _prep_inputs(batch: np.ndarray, W1: np.ndarray, W2: np.ndarray):
    import ml_dtypes

    bf16 = ml_dtypes.bfloat16
    fp8 = ml_dtypes.float8_e3m4
    wc = W2.astype(np.float64) @ W1.astype(np.float64)
    wc = wc / XS  # undo the host pre-scale of x
    # [112, 7*10]: wct[p, c*10+o] = Wc[o, 112c + p]
    wct = np.ascontiguousarray(
        wc.T.reshape(NDC, DC, H2).transpose(1, 0, 2).reshape(DC, NDC * H2)
    ).astype(bf16)

    xq = (batch * np.float32(XS)).astype(fp8)  # one pass over the f32 data

    # full b's 0..95: [8, 12, 2, 112, 7, 1000]: core, b, t-half,
    # d%112 (partitions), d-chunk, t-within-half
    xt = np.ascontiguousarray(
        xq[: NCORES * BPF]
        .reshape(NCORES, BPF, 2, T // 2, NDC, DC)
        .transpose(0, 1, 2, 5, 4, 3)
    )

    # T-segments of b's 96..99: core c gets b 96+c//2, half c%2.
    # Lower half: t 0..1152 (host keeps t<1024).  Upper half: t
    # 896..2048 (first 128 are recursion warm-up; host keeps t>=1024).
    seg = np.zeros((NCORES, TS, DIN), fp8)
    for c in range(NCORES):
        be = NCORES * BPF + c // 2
        if c % 2 == 0:
            seg[c] = xq[be, 0:TS]
        else:
            seg[c, 0 : T - (SEG_LO - SEG_W0)] = xq[be, SEG_LO - SEG_W0 :]
    xs = np.ascontiguousarray(
        seg.reshape(NCORES, 2, TS // 2, NDC, DC).transpose(0, 1, 4, 3, 2)
    )
    return xt, xs, wct


def kernel(batch: np.ndarray, W1: np.ndarray, W2: np.ndarray) -> np.ndarray:
    from concourse import bass_utils

    if "nc" not in _CACHE:
        _CACHE["nc"] = _build()
    nc = _CACHE["nc"]

    xt, xs, wct = _prep_inputs(batch, W1, W2)
    in_maps = [
        {"xT": xt[i], "xS": xs[i], "wct": wct} for i in range(NCORES)
    ]
    res = bass_utils.run_bass_kernel_spmd(
        nc, in_maps, core_ids=list(range(NCORES)), **_CACHE.get("run_kwargs", {})
    )
    _CACHE["last_result"] = res

    out = np.empty((B, T, H2), np.float32)
    for c in range(NCORES):
        vo = res.results[c]["vout"].astype(np.float32)  # [130, 2000]
        out[c * BPF : (c + 1) * BPF] = (
            vo[: BPF * H2].reshape(BPF, H2, T).transpose(0, 2, 1)
        )
        segv = vo[BPF * H2 :]  # [10, 2000]; valid cols 0..TS
        be = NCORES * BPF + c // 2
        if c % 2 == 0:
            out[be, 0:SEG_LO] = segv[:, 0:SEG_LO].T
        else:
            out[be, SEG_LO:T] = segv[:, SEG_W0 : SEG_W0 + (T - SEG_LO)].T
    return out
